# revision 5
# baseline (speedup 1.0000x reference)
"""Trainium2 Bass kernel for nn_CausalSelfAttention_37417755083187.

Full-input contract: kernel(**inputs) takes the unsharded fp32 inputs and
returns the full [B, T, C] fp32 output.  Sharding strategy: 8 cores =
(2 batches) x (4 head-groups of 4 heads).  The host-side shard step also
picks the on-device layout: x is shipped transposed [C, T] in bf16 (the
TensorE contraction needs channels on partitions), weights/ve/cos/sin are
shipped bf16.  Each core computes a partial projection output (row-split
Wproj) and the host sums the 4 partials per batch.

Per-core pipeline (bf16 matmuls, fp32 PSUM accumulation).  Engine roles:
ACT runs *only* the softmax EXPs (psum copies, DMA issues, sqrt were all
moved off its strict FIFO so EXP never queues behind a blocker); DVE runs
the RoPE/RMS epilogues with a reciprocal+Newton rsqrt (seeded affine in
1/m, 3 iterations), static-mask multiplies for the causal diagonal, and
half the proj psum drains; GpSimd runs the v-gate epilogue, the other
half of the drains and all output stores; Sync runs the q/k/y transposes
and the streamed x/ve input DMAs.  The schedule interleaves the next
chunk's QKV tiles and the previous chunk's projection blocks *between*
each head's score and AV matmuls, so the EXP latency of head h is hidden
behind ~4-10us of independent PE work instead of stalling the PE.
"""

import os
import sys

sys.path.insert(0, "/opt/trn_rl_repo")

from contextlib import ExitStack

import numpy as np

import concourse.bass as bass
import concourse.mybir as mybir
import concourse.tile as tile
from concourse import bacc
from concourse.alu_op_type import AluOpType as alu

F32 = mybir.dt.float32
BF16 = mybir.dt.bfloat16
AF = mybir.ActivationFunctionType

# Problem constants (hardcoded per harness contract)
B, T, C = 2, 2048, 2048
NH = 16
HD = 128
D2 = HD // 2  # 64
GATE = 32
EPS = 1e-6
N_CORES = 8
N_GROUPS = 4          # head-groups (tensor parallel)
NHC = NH // N_GROUPS  # heads per core = 4


def _rsqrt_consts(m0):
    """Affine-in-1/m seed for the DVE Newton rsqrt, fit over [m0/2.5, 2.5*m0]."""
    u_lo, u_hi = 1.0 / (2.5 * m0), 2.5 / m0
    s_lo, s_hi = float(np.sqrt(u_lo)), float(np.sqrt(u_hi))
    slope = (s_hi - s_lo) / (u_hi - u_lo)
    return slope, s_lo - slope * u_lo


def build_nc(T_=T, C_=C, NHC_=NHC, num_devices=N_CORES):
    """Build the Bass program for one core (SPMD: all cores run this)."""
    NQ = NHC_ * HD          # per-core qkv width
    TT = T_ // 128          # token tiles
    CT = C_ // 128          # channel tiles
    NCH = T_ // 512         # 512-token chunks
    VW = 130                # per-head v width: 128 v + 1 ones + 1 pad

    nc = bacc.Bacc(
        "TRN2",
        target_bir_lowering=False,
        debug=False,
        enable_asserts=False,
        num_devices=num_devices,
    )

    xT_d = nc.dram_tensor("xT_s", [C_, T_], BF16, kind="ExternalInput").ap()
    ve_d = nc.dram_tensor("ve_s", [T_, NQ], BF16, kind="ExternalInput").ap()
    cos_d = nc.dram_tensor("cos_s", [T_, D2], BF16, kind="ExternalInput").ap()
    sin_d = nc.dram_tensor("sin_s", [T_, D2], BF16, kind="ExternalInput").ap()
    wq_d = nc.dram_tensor("wq_s", [C_, NQ], BF16, kind="ExternalInput").ap()
    wk_d = nc.dram_tensor("wk_s", [C_, NQ], BF16, kind="ExternalInput").ap()
    wv_d = nc.dram_tensor("wv_s", [C_, NQ], BF16, kind="ExternalInput").ap()
    wg_d = nc.dram_tensor("wg_s", [GATE, NHC_], BF16, kind="ExternalInput").ap()
    wp_d = nc.dram_tensor("wp_s", [NQ, C_], BF16, kind="ExternalInput").ap()
    out_d = nc.dram_tensor("out_s", [T_, C_], F32, kind="ExternalOutput").ap()

    with ExitStack() as ctx:
        tc = ctx.enter_context(tile.TileContext(nc))
        pp = ctx.enter_context(tc.tile_pool(name="persist", bufs=1))
        pw = ctx.enter_context(tc.tile_pool(name="work", bufs=2))
        psQ = ctx.enter_context(tc.tile_pool(name="psQ", bufs=4, space="PSUM"))
        psS = ctx.enter_context(tc.tile_pool(name="psS", bufs=2, space="PSUM"))

        kT = pp.tile([128, NHC_, T_], BF16, name="kT")   # [d, h, t] all chunks
        vext = pp.tile([128, TT, NHC_ * VW], BF16, name="vext")
        g_all = pp.tile([128, TT, NHC_], F32, name="g_all")
        cos_bf = pp.tile([128, TT, D2], BF16, name="cos_bf")
        sin_bf = pp.tile([128, TT, D2], BF16, name="sin_bf")
        wgate_b = pp.tile([GATE, NHC_], BF16, name="wgate_b")
        wq_b = pp.tile([128, CT, NQ], BF16, name="wq_b")
        wk_b = pp.tile([128, CT, NQ], BF16, name="wk_b")
        wv_b = pp.tile([128, CT, NQ], BF16, name="wv_b")
        wp_b = pp.tile([128, NHC_, C_], BF16, name="wp_b")
        # static causal masks for the diagonal 512x512 block (chunk-invariant)
        maskA = pp.tile([128, 2, 512], BF16, name="maskA")
        maskB = pp.tile([128, 2, 256], BF16, name="maskB")

        vext_v = vext.rearrange("p t (h c) -> p t h c", c=VW)
        xT_r = xT_d.rearrange("(a p) t -> p a t", p=128)

        # PE warmup: dense dummy matmuls during the DMA preamble keep the
        # HAM clock-gate warm so the first real MMs run at 2.4 GHz.
        wu = pp.tile([128, 128], BF16, name="wu")
        nc.vector.memset(wu, 0.0)
        wu_ps = psQ.tile([128, 512], F32, tag="qkv")
        for _ in range(40):
            nc.tensor.matmul(wu_ps[:, 0:128], wu, wu, start=True, stop=True)
        # preload the EXP table during the DMA preamble (table load is 1.3us)
        exw = pp.tile([128, 8], F32, name="exw")
        nc.vector.memset(exw, 0.0)
        nc.scalar.activation(exw, exw, AF.Exp)

        # ---- constant / weight loads (bf16 direct from host-cast inputs) ----
        # Streamed inputs (xT halves) go on Sync HWDGE; bulk weights go on
        # GpSimd SWDGE so the ACT queue never sees a DMA issue.  First
        # chunk's critical inputs lead: wgate, c-tile 0 of wq/wk/wv, xTc0.
        nc.sync.dma_start(wgate_b, wg_d)
        xTc0 = pw.tile([128, CT, 256], BF16, tag="xT", bufs=2)
        nc.sync.dma_start(xTc0[:, 0:4, :], xT_r[:, 0:4, 0:256])
        for wd, wb in ((wq_d, wq_b), (wk_d, wk_b), (wv_d, wv_b)):
            nc.gpsimd.dma_start(
                wb[:, 0:1, :],
                wd.rearrange("(a p) n -> p a n", p=128)[:, 0:1, :])
        nc.sync.dma_start(xTc0[:, 4:CT, :], xT_r[:, 4:CT, 0:256])
        for wd, wb in ((wq_d, wq_b), (wk_d, wk_b), (wv_d, wv_b)):
            nc.gpsimd.dma_start(
                wb[:, 1:4, :],
                wd.rearrange("(a p) n -> p a n", p=128)[:, 1:4, :])
        nc.sync.dma_start(cos_bf, cos_d.rearrange("(a p) d -> p a d", p=128))
        nc.sync.dma_start(sin_bf, sin_d.rearrange("(a p) d -> p a d", p=128))
        nq = CT // 4
        for qtr in range(1, 4):
            for wd, wb in ((wq_d, wq_b), (wk_d, wk_b), (wv_d, wv_b)):
                nc.gpsimd.dma_start(
                    wb[:, qtr * nq:(qtr + 1) * nq, :],
                    wd.rearrange("(a p) n -> p a n", p=128)[:, qtr * nq:(qtr + 1) * nq, :])
        nc.gpsimd.memset(vext, 0.0)
        nc.gpsimd.memset(vext_v[:, :, :, 128:129], 1.0)
        # static diagonal masks: keep where q >= k + 128*a (A) and, for the
        # upper diagonal pair, q_local >= k + 128*a with q_local = q - 256 (B)
        nc.vector.memset(maskA, 1.0)
        nc.vector.memset(maskB, 1.0)
        nc.gpsimd.affine_select(
            out=maskA, in_=maskA, pattern=[[-128, 2], [1, 512]],
            compare_op=alu.is_ge, fill=0.0, base=0, channel_multiplier=-1)
        nc.gpsimd.affine_select(
            out=maskB, in_=maskB, pattern=[[-128, 2], [1, 256]],
            compare_op=alu.is_ge, fill=0.0, base=0, channel_multiplier=-1)

        # DVE rsqrt constants (q row a=0 folds the /sqrt(HD); k row a=1)
        m0q = float(HD * C_ * 0.02 * 0.02)
        sl_q, ic_q = _rsqrt_consts(m0q)
        sl_k, ic_k = _rsqrt_consts(m0q / HD)

        # --------- A-section emitter (one token-tile) ---------
        xTc_cache = {0: xTc0}
        qT_tiles = {}

        def emit_A(t):
            ch_t, t4 = divmod(t, 4)
            if t4 == 0:
                qT_tiles[ch_t] = pw.tile([128, NHC_, 512], BF16, tag="qT",
                                         bufs=2, name=f"qT_{ch_t}")
            qT = qT_tiles[ch_t]
            if t % 2 == 0 and t > 0:
                xTc = pw.tile([128, CT, 256], BF16, tag="xT", bufs=2,
                              name=f"xTc_{t//2}")
                nc.sync.dma_start(xTc, xT_r[:, :, t * 128:t * 128 + 256])
                xTc_cache[t // 2] = xTc
            xTc = xTc_cache[t // 2]
            tsl = slice((t % 2) * 128, (t % 2) * 128 + 128)

            # gate: u = (x[:, :32] @ (Wg/2)) ; gate = 1 + tanh(u) via series
            gps = psQ.tile([128, NQ], F32, tag="qkv")
            nc.tensor.matmul(gps[:, 0:NHC_], xTc[0:GATE, 0, tsl], wgate_b,
                             start=True, stop=True)
            gu = pw.tile([128, NHC_], F32, tag="gu", bufs=2)
            nc.vector.tensor_copy(gu, gps[:, 0:NHC_])
            ga = pw.tile([128, NHC_], F32, tag="ga", bufs=2)
            nc.vector.tensor_mul(ga, gu, gu)          # u^2
            gb = pw.tile([128, NHC_], F32, tag="gb", bufs=2)
            nc.vector.tensor_mul(gb, ga, gu)          # u^3
            gc = pw.tile([128, NHC_], F32, tag="gc", bufs=2)
            nc.vector.scalar_tensor_tensor(out=gc, in0=gb, scalar=-1.0 / 3.0,
                                           in1=gu, op0=alu.mult, op1=alu.add)
            ge = pw.tile([128, NHC_], F32, tag="ge", bufs=2)
            nc.vector.tensor_mul(ge, ga, gb)          # u^5
            gf = pw.tile([128, NHC_], F32, tag="gf", bufs=2)
            nc.vector.scalar_tensor_tensor(out=gf, in0=ge, scalar=2.0 / 15.0,
                                           in1=gc, op0=alu.mult, op1=alu.add)
            nc.vector.tensor_scalar_add(g_all[:, t, :], gf, 1.0)

            # QKV matmuls, interleaved over c so each xT ldweights feeds 3 MMs
            qps = psQ.tile([128, NQ], F32, tag="qkv")
            kps = psQ.tile([128, NQ], F32, tag="qkv")
            vps = psQ.tile([128, NQ], F32, tag="qkv")
            for c in range(CT):
                lhs = xTc[:, c, tsl]
                st, sp = (c == 0), (c == CT - 1)
                nc.tensor.matmul(qps, lhs, wq_b[:, c, :], start=st, stop=sp)
                nc.tensor.matmul(kps, lhs, wk_b[:, c, :], start=st, stop=sp)
                nc.tensor.matmul(vps, lhs, wv_b[:, c, :], start=st, stop=sp)

            # psum -> sbuf drains on DVE (GPSIMD cannot access PSUM; ACT
            # stays EXP-only)
            qkb = pw.tile([128, 2, NQ], BF16, tag="qkb", bufs=2)
            nc.vector.tensor_copy(qkb[:, 0, :], qps)
            nc.vector.tensor_copy(qkb[:, 1, :], kps)

            # v epilogue (DVE: the per-partition-scalar STT only lowers there)
            vet = pw.tile([128, NQ], BF16, tag="ve", bufs=2)
            nc.sync.dma_start(vet, ve_d[bass.ts(t, 128), :])
            for h in range(NHC_):
                nc.vector.scalar_tensor_tensor(
                    out=vext_v[:, t, h, 0:128],
                    in0=vet[:, bass.ts(h, 128)],
                    scalar=g_all[:, t, h:h + 1],
                    in1=vps[:, bass.ts(h, 128)],
                    op0=alu.mult, op1=alu.add)

            # q/k epilogue: RoPE + RMS-norm + transpose
            qk4 = qkb.rearrange("p a (h x d) -> p a h x d", h=NHC_, x=2)
            z1 = qk4[:, :, :, 0, :]
            z2 = qk4[:, :, :, 1, :]
            cb = cos_bf[:, t, :].unsqueeze(1).unsqueeze(1) \
                .broadcast_to([128, 2, NHC_, D2])
            sb = sin_bf[:, t, :].unsqueeze(1).unsqueeze(1) \
                .broadcast_to([128, 2, NHC_, D2])
            rot = pw.tile([128, 2, NQ], BF16, tag="rot", bufs=2)
            rot4 = rot.rearrange("p a (h x d) -> p a h x d", h=NHC_, x=2)
            t1 = pw.tile([128, 2, NHC_, D2], BF16, tag="t1", bufs=2)
            t2 = pw.tile([128, 2, NHC_, D2], BF16, tag="t2", bufs=2)
            nc.vector.tensor_mul(t1, z1, cb)
            nc.vector.tensor_mul(t2, z2, sb)
            nc.vector.tensor_add(rot4[:, :, :, 0, :], t1, t2)
            nc.vector.tensor_mul(t1, z2, cb)
            nc.vector.tensor_mul(t2, z1, sb)
            nc.vector.tensor_sub(rot4[:, :, :, 1, :], t1, t2)

            # RMS stats with the eps/mean folds:
            #   q: m = sum(rot^2) + HD*eps      (rsqrt -> combined /sqrt(HD))
            #   k: m = sum(rot^2)/HD + eps
            sums = pw.tile([128, 2, NHC_], F32, tag="sums", bufs=2)
            sq = pw.tile([128, 2, NHC_, HD], BF16, tag="sq", bufs=1)
            rot_h = rot.rearrange("p a (h d) -> p a h d", h=NHC_)
            nc.vector.tensor_mul(sq, rot_h, rot_h)
            nc.vector.reduce_sum(sums, sq, axis=mybir.AxisListType.X)
            nc.vector.tensor_scalar_add(sums[:, 0, :], sums[:, 0, :],
                                        float(HD) * EPS)
            nc.vector.tensor_scalar(out=sums[:, 1, :], in0=sums[:, 1, :],
                                    scalar1=1.0 / HD, scalar2=EPS,
                                    op0=alu.mult, op1=alu.add)
            # rsqrt fully on DVE: affine-in-1/m seed + 3 Newton iterations
            # (max rel err 1.6e-6 over [m0/4, 4*m0]; no ACT table swap)
            r0 = pw.tile([128, 2, NHC_], F32, tag="r0", bufs=2)
            nc.vector.reciprocal(r0, sums)
            nc.vector.tensor_scalar(out=r0[:, 0, :], in0=r0[:, 0, :],
                                    scalar1=sl_q, scalar2=ic_q,
                                    op0=alu.mult, op1=alu.add)
            nc.vector.tensor_scalar(out=r0[:, 1, :], in0=r0[:, 1, :],
                                    scalar1=sl_k, scalar2=ic_k,
                                    op0=alu.mult, op1=alu.add)
            n1 = pw.tile([128, 2, NHC_], F32, tag="n1", bufs=2)
            for _ in range(3):
                nc.vector.tensor_mul(n1, r0, r0)
                nc.vector.tensor_mul(n1, n1, sums)
                nc.vector.tensor_scalar(out=n1, in0=n1, scalar1=-0.5,
                                        scalar2=1.5, op0=alu.mult, op1=alu.add)
                nc.vector.tensor_mul(r0, r0, n1)
            for a in range(2):
                for h in range(NHC_):
                    sl = rot[:, a, bass.ts(h, HD)]
                    nc.vector.tensor_scalar_mul(sl, sl, r0[:, a, h:h + 1])
            nc.sync.dma_start_transpose(qT[:, :, bass.ts(t4, 128)], rot[:, 0, :])
            nc.sync.dma_start_transpose(kT[:, :, bass.ts(t, 128)], rot[:, 1, :])

        # --------- B-section: scores+exp for one (chunk, head) ---------
        def emit_scores(ch, h):
            qT = qT_tiles[ch]
            n_tk = 4 * (ch + 1)
            P_all = pw.tile([128, TT, 512], BF16, tag="P", bufs=2)
            for p in range(n_tk // 2):
                s_ps = psS.tile([128, 2, 512], F32, tag="s")
                for s2 in (0, 1):
                    i = 2 * p + s2
                    nc.tensor.matmul(
                        s_ps[:, s2, :],
                        kT[:, h, bass.ts(i, 128)],
                        qT[:, h, :],
                        start=True, stop=True)
                sl2 = slice(2 * p, 2 * p + 2)
                if p == n_tk // 2 - 1:
                    # last diagonal pair: tq < 256 is entirely masked --
                    # zero it and exp/mask only the valid half
                    nc.vector.memset(P_all[:, sl2, 0:256], 0.0)
                    nc.scalar.activation(P_all[:, sl2, 256:],
                                         s_ps[:, :, 256:], AF.Exp)
                    nc.vector.tensor_mul(P_all[:, sl2, 256:],
                                         P_all[:, sl2, 256:], maskB)
                elif p == n_tk // 2 - 2:
                    nc.scalar.activation(P_all[:, sl2, :], s_ps, AF.Exp)
                    nc.vector.tensor_mul(P_all[:, sl2, :],
                                         P_all[:, sl2, :], maskA)
                else:
                    nc.scalar.activation(P_all[:, sl2, :], s_ps, AF.Exp)
            return P_all

        # --------- AV for one (chunk, head) ---------
        def emit_av(ch, h, P_all, yn, tail_cb=None):
            for q4 in range(4):
                tqt = 4 * ch + q4
                y_ps = psQ.tile([128, 512], F32, tag="qkv")
                for i in range(tqt + 1):
                    nc.tensor.matmul(
                        y_ps[:, 0:HD + 1],
                        P_all[:, i, bass.ts(q4, 128)],
                        vext_v[:, i, h, 0:HD + 1],
                        start=(i == 0), stop=(i == tqt))
                # normalize on DVE: 1/den then per-partition multiply
                dr = pw.tile([128, 1], F32, tag="dr", bufs=2)
                nc.vector.reciprocal(dr, y_ps[:, HD:HD + 1])
                nc.vector.tensor_scalar_mul(yn[:, q4, bass.ts(h, HD)],
                                            y_ps[:, 0:HD], dr)
                if tail_cb is not None:
                    tail_cb(q4)

        # --------- projection block for one token-tile ---------
        def emit_C_t4(yT, ch, t4):
            t = ch * 4 + t4
            for g in range(2):
                ob = pw.tile([128, 1024], F32, tag="ob", bufs=2)
                for c2 in range(2):
                    c4 = g * 2 + c2
                    o_ps = psQ.tile([128, 512], F32, tag="qkv")
                    for hh in range(NHC_):
                        nc.tensor.matmul(o_ps, yT[:, hh, t4, :],
                                         wp_b[:, hh, bass.ts(c4, 512)],
                                         start=(hh == 0), stop=(hh == NHC_ - 1))
                    nc.vector.tensor_copy(ob[:, bass.ts(c2, 512)], o_ps)
                nc.gpsimd.dma_start(
                    out_d[bass.ts(t, 128), g * 1024:(g + 1) * 1024], ob)

        # --------- pipelined schedule ---------
        # Preamble: all of chunk 0's tiles + chunk 1's first tile.
        for t in range(5):
            emit_A(t)
        nc.gpsimd.dma_start(wp_b, wp_d.rearrange("(h p) c -> p h c", p=128))

        yn_prev = None
        yT_prev = None
        for ch in range(NCH):
            yn = pw.tile([128, 4, NQ], BF16, tag="yn", bufs=2, name=f"yn_{ch}")
            if yn_prev is not None:
                yT_prev = pw.tile([128, NHC_, 4, 128], BF16, tag="yT", bufs=2)
                for t4 in range(4):
                    nc.sync.dma_start_transpose(yT_prev[:, :, t4, :],
                                                yn_prev[:, t4, :])
            pending_av = None
            for h in range(NHC_):
                P_all = emit_scores(ch, h)
                # exp(h) window: C-block (ch>0) or an A-tile (ch==0), plus
                # the previous head's AV
                if yn_prev is not None:
                    emit_C_t4(yT_prev, ch - 1, h)
                if pending_av is not None:
                    pending_av()
                if ch + 1 < NCH:
                    if h < 3:
                        emit_A((ch + 1) * 4 + h + 1)
                    elif ch + 2 < NCH:
                        emit_A((ch + 2) * 4)
                hh, PP = h, P_all
                if ch == NCH - 1 and h == NHC_ - 1:
                    # tail: interleave the last chunk's projection with its
                    # final AV round (transpose each q4 row as it lands)
                    yT3 = pw.tile([128, NHC_, 4, 128], BF16, tag="yT", bufs=2)

                    def tail_cb(q4, yT3=yT3, yn=yn, ch=ch):
                        nc.sync.dma_start_transpose(yT3[:, :, q4, :],
                                                    yn[:, q4, :])
                        if q4 >= 1:
                            emit_C_t4(yT3, ch, q4 - 1)

                    pending_av = (lambda hh=hh, PP=PP:
                                  emit_av(ch, hh, PP, yn, tail_cb))
                    final_yT = yT3
                else:
                    pending_av = (lambda hh=hh, PP=PP:
                                  emit_av(ch, hh, PP, yn))
            pending_av()
            yn_prev = yn
        emit_C_t4(final_yT, NCH - 1, 3)

    nc.compile()
    return nc


def shard_inputs(inputs):
    """Full fp32 inputs -> list of 8 per-core input maps (bf16 device layout)."""
    import ml_dtypes

    bf16 = ml_dtypes.bfloat16
    x = np.asarray(inputs["x"], np.float32)
    ve = np.asarray(inputs["ve"], np.float32)
    cos = np.asarray(inputs["cos"], np.float32).reshape(T, D2)
    sin = np.asarray(inputs["sin"], np.float32).reshape(T, D2)
    wq = np.asarray(inputs["Wq"], np.float32)
    wk = np.asarray(inputs["Wk"], np.float32)
    wv = np.asarray(inputs["Wv"], np.float32)
    wg = np.asarray(inputs["Wgate"], np.float32)
    wp = np.asarray(inputs["Wproj"], np.float32)

    NQ = NHC * HD
    cos_b = cos.astype(bf16)
    sin_b = sin.astype(bf16)
    xT = [np.ascontiguousarray(x[b].T.astype(bf16)) for b in range(B)]
    maps = []
    for core in range(N_CORES):
        b, g = divmod(core, N_GROUPS)
        sl = slice(g * NQ, (g + 1) * NQ)
        maps.append({
            "xT_s": xT[b],
            "ve_s": np.ascontiguousarray(ve[b][:, sl].astype(bf16)),
            "cos_s": cos_b,
            "sin_s": sin_b,
            "wq_s": np.ascontiguousarray(wq[:, sl].astype(bf16)),
            "wk_s": np.ascontiguousarray(wk[:, sl].astype(bf16)),
            "wv_s": np.ascontiguousarray(wv[:, sl].astype(bf16)),
            "wg_s": np.ascontiguousarray((wg[:, g * NHC:(g + 1) * NHC] * 0.5).astype(bf16)),
            "wp_s": np.ascontiguousarray(wp[sl, :].astype(bf16)),
        })
    return maps


_NC_CACHE = {}


def _get_nc():
    if "nc" not in _NC_CACHE:
        _NC_CACHE["nc"] = build_nc()
    return _NC_CACHE["nc"]


def kernel(**inputs) -> np.ndarray:
    from concourse.bass_utils import run_bass_kernel_spmd

    nc = _get_nc()
    in_maps = shard_inputs(inputs)
    res = run_bass_kernel_spmd(nc, in_maps, list(range(N_CORES)))
    out = np.zeros((B, T, C), np.float32)
    for core in range(N_CORES):
        b = core // N_GROUPS
        out[b] += res.results[core]["out_s"]
    return out
